# revision 24
# baseline (speedup 1.0000x reference)
"""Trainium2 Bass kernel for CornerBoundingBoxEMDLoss.

For each sample: 8x8 pairwise corner distances, then exact min-cost perfect
matching via meet-in-the-middle (identical math to the brute-force 8! GEMM):

  min over perms = min over 70 4-subsets T of
      (min assignment of preds {0,1,2,3} onto T)
    + (min assignment of preds {4,5,6,7} onto complement(T))

Pipeline (all GEMMs fp16 single-pass; data DMA lands before the table DMA):
  phase1: diff via one-hot selection GEMMs over [predT; -targT]  (PE),
          square (Act), coord-sum GEMMs in sample-halves (PE),
          sqrt in sample-halves -> distT fp16 (Act)
  L1:     pred-pair x target-pair costs, both orderings side-by-side in a
          single-bank PSUM tile per sample-half; one fused strided min
          reduce per half (DVE) -> m_t [112, 512]
  L2:     quad costs via one-hot GEMM [112 -> 840] per 128-sample chunk;
          min-of-6 reduce straight from PSUM (DVE)
  L3:     A+B adds (Pool; last chunk on DVE), one fused min-over-70 for
          all chunks -> loss [128, 4]
  out:    PE transpose [128, 64] -> [64, 128] fp32, one 2KB DMA

Dummy square/sqrt activations at kernel start pull both ACT_TABLE_LOADs
into the DMA-wait window; a couple of warmup matmuls keep the PE awake.
Data-parallel across 8 NeuronCores: 512 samples per core.
"""

import itertools

import numpy as np

import concourse.bacc as bacc
import concourse.mybir as mybir
import concourse.tile as tile

N_CORES = 8
B_TOTAL = 4096
B_CORE = B_TOTAL // N_CORES          # 512
N_CHUNKS = 4
CHUNK = B_CORE // N_CHUNKS           # 128

F32 = mybir.dt.float32
F16 = mybir.dt.float16

MIN_INIT = 1.0e30
N_WARMUP = 3                          # PE warmup matmuls during DMA wait

# packed input buffers (fp16)
# dbuf [48, 704]: data + phase-1 diff selection tables (lands first)
D_DATA = 0        # [48, 512]  predT rows 0:24, -targT rows 24:48
D_A1 = 512        # [48, 128]  diff selection, coords c in {0,1}
D_A2 = 640        # [48, 64]   diff selection, coord c = 2
W_D = 704
# tbuf [128, 1192]: remaining tables
T_CS1 = 0         # [128, 64]  coord-sum for sq1
T_L2 = 64         # [112, 840] quad tables (A cols 0:420, B cols 420:840)
T_L1 = 904        # [64, 224]  pair tables, both orderings
T_CS2 = 1128      # [64, 64]   coord-sum for sq2 (identity)
W_T = 1192


def _build_tables():
    """Host-side constant tables, all fp16-exact (0/1 entries)."""
    a1 = np.zeros((48, 128), dtype=np.float16)
    a2 = np.zeros((48, 64), dtype=np.float16)
    cs1 = np.zeros((128, 64), dtype=np.float16)
    cs2 = np.eye(64, dtype=np.float16)
    for i in range(8):
        for j in range(8):
            q = i * 8 + j
            for c in (0, 1):
                a1[i * 3 + c, c * 64 + q] = 1
                a1[24 + j * 3 + c, c * 64 + q] = 1
                cs1[c * 64 + q, q] = 1
            a2[i * 3 + 2, q] = 1
            a2[24 + j * 3 + 2, q] = 1

    pairs = list(itertools.combinations(range(8), 2))            # 28
    pair_idx = {p: i for i, p in enumerate(pairs)}
    subs4 = list(itertools.combinations(range(8), 4))            # 70
    pred_pairs = [(0, 1), (2, 3), (4, 5), (6, 7)]

    l1t = np.zeros((64, 224), dtype=np.float16)
    for q, (i0, i1) in enumerate(pred_pairs):
        for p, (a, b) in enumerate(pairs):
            col = q * 28 + p
            l1t[i0 * 8 + a, col] = 1
            l1t[i1 * 8 + b, col] = 1
            l1t[i0 * 8 + b, 112 + col] = 1
            l1t[i1 * 8 + a, 112 + col] = 1

    l2t = np.zeros((112, 840), dtype=np.float16)
    for t, T in enumerate(subs4):
        for s, S in enumerate(itertools.combinations(T, 2)):
            rest = tuple(sorted(set(T) - set(S)))
            l2t[0 * 28 + pair_idx[S], t * 6 + s] = 1
            l2t[1 * 28 + pair_idx[rest], t * 6 + s] = 1
        TB = tuple(sorted(set(range(8)) - set(T)))               # complement
        for s, S in enumerate(itertools.combinations(TB, 2)):
            rest = tuple(sorted(set(TB) - set(S)))
            l2t[2 * 28 + pair_idx[S], 420 + t * 6 + s] = 1
            l2t[3 * 28 + pair_idx[rest], 420 + t * 6 + s] = 1

    return a1, a2, cs1, cs2, l1t, l2t


def make_in_maps(pred_corners: np.ndarray, target_corners: np.ndarray):
    """Pack per-core fp16 input buffers (data-first DMA + tables DMA)."""
    a1, a2, cs1, cs2, l1t, l2t = _build_tables()
    ident32 = np.eye(128, dtype=np.float32)
    pred = np.ascontiguousarray(pred_corners, dtype=np.float32).reshape(B_TOTAL, 24)
    targ = np.ascontiguousarray(target_corners, dtype=np.float32).reshape(B_TOTAL, 24)

    tbuf = np.zeros((128, W_T), dtype=np.float16)
    tbuf[0:128, T_CS1:T_CS1 + 64] = cs1
    tbuf[0:112, T_L2:T_L2 + 840] = l2t
    tbuf[0:64, T_L1:T_L1 + 224] = l1t
    tbuf[0:64, T_CS2:T_CS2 + 64] = cs2

    in_maps = []
    for k in range(N_CORES):
        sl = slice(k * B_CORE, (k + 1) * B_CORE)
        dbuf = np.zeros((48, W_D), dtype=np.float16)
        dbuf[0:24, D_DATA:D_DATA + B_CORE] = pred[sl].T.astype(np.float16)
        dbuf[24:48, D_DATA:D_DATA + B_CORE] = (-targ[sl].T).astype(np.float16)
        dbuf[0:48, D_A1:D_A1 + 128] = a1
        dbuf[0:48, D_A2:D_A2 + 64] = a2
        in_maps.append({"dbuf": dbuf, "tbuf": tbuf, "ident32": ident32})
    return in_maps


def build_nc():
    nc = bacc.Bacc("TRN2", target_bir_lowering=False, debug=False)

    dbuf_d = nc.dram_tensor("dbuf", [48, W_D], F16, kind="ExternalInput")
    tbuf_d = nc.dram_tensor("tbuf", [128, W_T], F16, kind="ExternalInput")
    id_d = nc.dram_tensor("ident32", [128, 128], F32, kind="ExternalInput")
    out_d = nc.dram_tensor("out", [B_CORE], F32, kind="ExternalOutput")

    AF = mybir.ActivationFunctionType
    ALU = mybir.AluOpType

    with tile.TileContext(nc) as tc:
        with (
            tc.tile_pool(name="consts", bufs=1) as cpool,
            tc.tile_pool(name="work", bufs=1) as wpool,
            tc.tile_pool(name="ps_x", bufs=2, space="PSUM") as psx,
            tc.tile_pool(name="ps_q", bufs=2, space="PSUM") as psq,
        ):
            DIN = cpool.tile([48, W_D], F16, tag="din")
            TIN = cpool.tile([128, W_T], F16, tag="tin")
            ID32 = cpool.tile([128, 128], F32, tag="id32")
            nc.sync.dma_start(DIN[:, :], dbuf_d[:, :])
            nc.sync.dma_start(TIN[:, :], tbuf_d[:, :])
            nc.sync.dma_start(ID32[:, :], id_d[:, :])

            data = DIN[0:48, D_DATA:D_DATA + B_CORE]
            a1 = DIN[0:48, D_A1:D_A1 + 128]
            a2 = DIN[0:48, D_A2:D_A2 + 64]
            cs1 = TIN[0:128, T_CS1:T_CS1 + 64]
            cs2 = TIN[0:64, T_CS2:T_CS2 + 64]
            l1a = TIN[0:64, T_L1:T_L1 + 112]
            l1b = TIN[0:64, T_L1 + 112:T_L1 + 224]
            l2a = TIN[0:112, T_L2:T_L2 + 420]
            l2b = TIN[0:112, T_L2 + 420:T_L2 + 840]

            # -- prologue during DMA wait: act table loads + PE p-state ramp
            warm = wpool.tile([128, 512], F16, tag="warm")
            nc.vector.memset(warm[:, :], 0.0)
            dummy = wpool.tile([1, 2], F16, tag="dummy")
            nc.scalar.activation(dummy[0:1, 0:1], warm[0:1, 0:1], AF.Square)
            nc.scalar.activation(dummy[0:1, 1:2], warm[0:1, 0:1], AF.Sqrt)
            psW = psq.tile([128, 1024], F32, tag="q")
            for _ in range(N_WARMUP):
                nc.tensor.matmul(psW[:, 0:384], warm[:, 0:128], warm[:, 0:384],
                                 start=True, stop=True)

            # -- phase 1: diff -> square -> coord-sum -> sqrt => distT fp16
            psD1 = psx.tile([128, 512], F32, tag="x")
            nc.tensor.matmul(psD1[:, :], a1, data, start=True, stop=True)
            psD2 = psx.tile([128, 512], F32, tag="x")
            nc.tensor.matmul(psD2[0:64, :], a2, data, start=True, stop=True)

            sq1 = wpool.tile([128, 512], F16, tag="sq1")
            sq2 = wpool.tile([64, 512], F16, tag="sq2")
            nc.scalar.activation(sq1[:, :], psD1[:, :], AF.Square)
            nc.scalar.activation(sq2[:, :], psD2[0:64, :], AF.Square)

            psE1 = psx.tile([64, 256], F32, tag="e")
            nc.tensor.matmul(psE1[:, :], cs1, sq1[:, 0:256],
                             start=True, stop=False)
            nc.tensor.matmul(psE1[:, :], cs2, sq2[:, 0:256],
                             start=False, stop=True)
            psE2 = psx.tile([64, 256], F32, tag="e")
            nc.tensor.matmul(psE2[:, :], cs1, sq1[:, 256:512],
                             start=True, stop=False)
            nc.tensor.matmul(psE2[:, :], cs2, sq2[:, 256:512],
                             start=False, stop=True)

            distT = wpool.tile([64, 512], F16, tag="distT")
            nc.scalar.activation(distT[:, 0:256], psE1[:, :], AF.Sqrt)
            nc.scalar.activation(distT[:, 256:512], psE2[:, :], AF.Sqrt)

            # -- L1 in sample-halves: per half, both orderings side-by-side
            # in one single-bank tile; fused strided min reduce per half
            m_t = wpool.tile([112, 512], F16, tag="m")
            psL1 = psx.tile([128, 512], F32, tag="x")
            nc.tensor.matmul(psL1[0:112, 0:256], l1a, distT[:, 0:256],
                             start=True, stop=True)
            nc.tensor.matmul(psL1[0:112, 256:512], l1b, distT[:, 0:256],
                             start=True, stop=True)
            nc.vector.tensor_reduce(
                m_t[:, 0:256],
                psL1[0:112, :].rearrange("p (k j) -> p j k", k=2),
                axis=mybir.AxisListType.X, op=ALU.min)
            psL2 = psx.tile([128, 512], F32, tag="x")
            nc.tensor.matmul(psL2[0:112, 0:256], l1a, distT[:, 256:512],
                             start=True, stop=True)
            nc.tensor.matmul(psL2[0:112, 256:512], l1b, distT[:, 256:512],
                             start=True, stop=True)
            nc.vector.tensor_reduce(
                m_t[:, 256:512],
                psL2[0:112, :].rearrange("p (k j) -> p j k", k=2),
                axis=mybir.AxisListType.X, op=ALU.min)

            # -- L2 + L3 per 128-sample chunk
            minall = wpool.tile([128, 560], F16, tag="minall")
            msum = wpool.tile([128, 280], F16, tag="msum")
            loss = wpool.tile([128, 64], F32, tag="loss")
            nc.vector.memset(loss[:, :], 0.0)

            for c in range(N_CHUNKS):
                sl = slice(c * CHUNK, (c + 1) * CHUNK)
                psQ = psq.tile([128, 1024], F32, tag="q")
                nc.tensor.matmul(psQ[:, 0:420], m_t[:, sl], l2a,
                                 start=True, stop=True)
                nc.tensor.matmul(psQ[:, 512:932], m_t[:, sl], l2b,
                                 start=True, stop=True)

                psv = (psQ[:, :].rearrange("p (k j) -> p k j", k=2)[:, :, 0:420]
                       .rearrange("p k (t s) -> p k t s", s=6))
                mout = minall[:, c * 140:(c + 1) * 140]
                nc.vector.tensor_reduce(
                    mout, psv, axis=mybir.AxisListType.X, op=ALU.min)

                # A+B add (Pool for c0-c2; DVE for c3 to shorten the tail)
                eng = nc.gpsimd if c < 3 else nc.vector
                eng.tensor_tensor(
                    msum[:, c * 70:(c + 1) * 70],
                    minall[:, c * 140:c * 140 + 70],
                    minall[:, c * 140 + 70:c * 140 + 140],
                    op=ALU.add)

            # one fused min-over-70 for all four chunks
            nc.vector.tensor_reduce(
                loss[:, 0:4], msum[:, :].rearrange("p (c t) -> p c t", c=4),
                axis=mybir.AxisListType.X, op=ALU.min)

            # -- finale: fp32 transpose [128,64] -> [64,128], one 2KB DMA
            psT = psx.tile([64, 256], F32, tag="e")
            nc.tensor.transpose(psT[:, 0:128], loss[:, :], ID32[:, :])
            outb = wpool.tile([4, 128], F32, tag="outb")
            nc.vector.tensor_copy(outb[:, :], psT[0:4, 0:128])
            nc.sync.dma_start(
                out_d[:].rearrange("(c p) -> c p", p=128), outb[:, :])

    nc.compile()
    return nc


_NC = None


def _get_nc():
    global _NC
    if _NC is None:
        _NC = build_nc()
    return _NC


def kernel(pred_corners: np.ndarray, target_corners: np.ndarray) -> np.ndarray:
    from concourse.bass_utils import run_bass_kernel_spmd

    nc = _get_nc()
    in_maps = make_in_maps(pred_corners, target_corners)
    res = run_bass_kernel_spmd(nc, in_maps, core_ids=list(range(N_CORES)))
    return np.concatenate([res.results[k]["out"] for k in range(N_CORES)])


# revision 25
# speedup vs baseline: 1.0352x; 1.0352x over previous
"""Trainium2 Bass kernel for CornerBoundingBoxEMDLoss.

For each sample: 8x8 pairwise corner distances, then exact min-cost perfect
matching via meet-in-the-middle (identical math to the brute-force 8! GEMM):

  min over perms = min over 70 4-subsets T of
      (min assignment of preds {0,1,2,3} onto T)
    + (min assignment of preds {4,5,6,7} onto complement(T))

Pipeline (all GEMMs fp16 single-pass; data DMA lands before the table DMA):
  phase1: diff via one-hot selection GEMMs over [predT; -targT]  (PE),
          square (Act), coord-sum GEMMs in sample-halves (PE),
          sqrt in sample-halves -> distT fp16 (Act)
  L1:     pred-pair x target-pair costs, both orderings side-by-side in a
          single-bank PSUM tile per sample-half; one fused strided min
          reduce per half (DVE) -> m_t [112, 512]
  L2:     quad costs via one-hot GEMM [112 -> 840] per 128-sample chunk;
          min-of-6 reduce straight from PSUM (DVE)
  L3:     A+B adds (Pool; last chunk on DVE), one fused min-over-70 for
          all chunks -> loss [128, 4]
  out:    PE transpose [128, 64] -> [64, 128] fp32, one 2KB DMA

Dummy square/sqrt activations at kernel start pull both ACT_TABLE_LOADs
into the DMA-wait window; a couple of warmup matmuls keep the PE awake.
Data-parallel across 8 NeuronCores: 512 samples per core.
"""

import itertools

import numpy as np

import concourse.bacc as bacc
import concourse.mybir as mybir
import concourse.tile as tile

N_CORES = 8
B_TOTAL = 4096
B_CORE = B_TOTAL // N_CORES          # 512
N_CHUNKS = 4
CHUNK = B_CORE // N_CHUNKS           # 128

F32 = mybir.dt.float32
F16 = mybir.dt.float16

MIN_INIT = 1.0e30
N_WARMUP = 3                          # PE warmup matmuls during DMA wait

# packed input buffers (fp16)
# dbuf [48, 704]: data + phase-1 diff selection tables (lands first)
D_DATA = 0        # [48, 512]  predT rows 0:24, -targT rows 24:48
D_A1 = 512        # [48, 128]  diff selection, coords c in {0,1}
D_A2 = 640        # [48, 64]   diff selection, coord c = 2
W_D = 704
# tbuf [128, 1192]: remaining tables
T_CS1 = 0         # [128, 64]  coord-sum for sq1
T_L2 = 64         # [112, 840] quad tables (A cols 0:420, B cols 420:840)
T_L1 = 904        # [64, 224]  pair tables, both orderings
T_CS2 = 1128      # [64, 64]   coord-sum for sq2 (identity)
W_T = 1192


def _build_tables():
    """Host-side constant tables, all fp16-exact (0/1 entries)."""
    a1 = np.zeros((48, 128), dtype=np.float16)
    a2 = np.zeros((48, 64), dtype=np.float16)
    cs1 = np.zeros((128, 64), dtype=np.float16)
    cs2 = np.eye(64, dtype=np.float16)
    for i in range(8):
        for j in range(8):
            q = i * 8 + j
            for c in (0, 1):
                a1[i * 3 + c, c * 64 + q] = 1
                a1[24 + j * 3 + c, c * 64 + q] = 1
                cs1[c * 64 + q, q] = 1
            a2[i * 3 + 2, q] = 1
            a2[24 + j * 3 + 2, q] = 1

    pairs = list(itertools.combinations(range(8), 2))            # 28
    pair_idx = {p: i for i, p in enumerate(pairs)}
    subs4 = list(itertools.combinations(range(8), 4))            # 70
    pred_pairs = [(0, 1), (2, 3), (4, 5), (6, 7)]

    l1t = np.zeros((64, 224), dtype=np.float16)
    for q, (i0, i1) in enumerate(pred_pairs):
        for p, (a, b) in enumerate(pairs):
            col = q * 28 + p
            l1t[i0 * 8 + a, col] = 1
            l1t[i1 * 8 + b, col] = 1
            l1t[i0 * 8 + b, 112 + col] = 1
            l1t[i1 * 8 + a, 112 + col] = 1

    l2t = np.zeros((112, 840), dtype=np.float16)
    for t, T in enumerate(subs4):
        for s, S in enumerate(itertools.combinations(T, 2)):
            rest = tuple(sorted(set(T) - set(S)))
            l2t[0 * 28 + pair_idx[S], t * 6 + s] = 1
            l2t[1 * 28 + pair_idx[rest], t * 6 + s] = 1
        TB = tuple(sorted(set(range(8)) - set(T)))               # complement
        for s, S in enumerate(itertools.combinations(TB, 2)):
            rest = tuple(sorted(set(TB) - set(S)))
            l2t[2 * 28 + pair_idx[S], 420 + t * 6 + s] = 1
            l2t[3 * 28 + pair_idx[rest], 420 + t * 6 + s] = 1

    return a1, a2, cs1, cs2, l1t, l2t


def make_in_maps(pred_corners: np.ndarray, target_corners: np.ndarray):
    """Pack per-core fp16 input buffers (data-first DMA + tables DMA)."""
    a1, a2, cs1, cs2, l1t, l2t = _build_tables()
    ident32 = np.eye(128, dtype=np.float32)
    pred = np.ascontiguousarray(pred_corners, dtype=np.float32).reshape(B_TOTAL, 24)
    targ = np.ascontiguousarray(target_corners, dtype=np.float32).reshape(B_TOTAL, 24)

    tbuf = np.zeros((128, W_T), dtype=np.float16)
    tbuf[0:128, T_CS1:T_CS1 + 64] = cs1
    tbuf[0:112, T_L2:T_L2 + 840] = l2t
    tbuf[0:64, T_L1:T_L1 + 224] = l1t
    tbuf[0:64, T_CS2:T_CS2 + 64] = cs2

    in_maps = []
    for k in range(N_CORES):
        sl = slice(k * B_CORE, (k + 1) * B_CORE)
        dbuf = np.zeros((48, W_D), dtype=np.float16)
        dbuf[0:24, D_DATA:D_DATA + B_CORE] = pred[sl].T.astype(np.float16)
        dbuf[24:48, D_DATA:D_DATA + B_CORE] = (-targ[sl].T).astype(np.float16)
        dbuf[0:48, D_A1:D_A1 + 128] = a1
        dbuf[0:48, D_A2:D_A2 + 64] = a2
        in_maps.append({"dbuf": dbuf, "tbuf": tbuf, "ident32": ident32})
    return in_maps


def build_nc():
    nc = bacc.Bacc("TRN2", target_bir_lowering=False, debug=False)

    dbuf_d = nc.dram_tensor("dbuf", [48, W_D], F16, kind="ExternalInput")
    tbuf_d = nc.dram_tensor("tbuf", [128, W_T], F16, kind="ExternalInput")
    id_d = nc.dram_tensor("ident32", [128, 128], F32, kind="ExternalInput")
    out_d = nc.dram_tensor("out", [B_CORE], F32, kind="ExternalOutput")
    wout_d = nc.dram_tensor("warmout", [1, 64], F16, kind="ExternalOutput")

    AF = mybir.ActivationFunctionType
    ALU = mybir.AluOpType

    with tile.TileContext(nc) as tc:
        with (
            tc.tile_pool(name="consts", bufs=1) as cpool,
            tc.tile_pool(name="work", bufs=1) as wpool,
            tc.tile_pool(name="ps_x", bufs=2, space="PSUM") as psx,
            tc.tile_pool(name="ps_q", bufs=2, space="PSUM") as psq,
        ):
            DIN = cpool.tile([48, W_D], F16, tag="din")
            TIN = cpool.tile([128, W_T], F16, tag="tin")
            ID32 = cpool.tile([128, 128], F32, tag="id32")
            nc.sync.dma_start(DIN[:, :], dbuf_d[:, :])
            nc.sync.dma_start(TIN[:, :], tbuf_d[:, :])
            nc.sync.dma_start(ID32[:, :], id_d[:, :])

            data = DIN[0:48, D_DATA:D_DATA + B_CORE]
            a1 = DIN[0:48, D_A1:D_A1 + 128]
            a2 = DIN[0:48, D_A2:D_A2 + 64]
            cs1 = TIN[0:128, T_CS1:T_CS1 + 64]
            cs2 = TIN[0:64, T_CS2:T_CS2 + 64]
            l1a = TIN[0:64, T_L1:T_L1 + 112]
            l1b = TIN[0:64, T_L1 + 112:T_L1 + 224]
            l2a = TIN[0:112, T_L2:T_L2 + 420]
            l2b = TIN[0:112, T_L2 + 420:T_L2 + 840]

            # -- prologue during DMA wait: act table loads + PE p-state ramp
            warm = wpool.tile([128, 512], F16, tag="warm")
            nc.vector.memset(warm[:, :], 0.0)
            dummy = wpool.tile([1, 2], F16, tag="dummy")
            nc.scalar.activation(dummy[0:1, 0:1], warm[0:1, 0:1], AF.Square)
            nc.scalar.activation(dummy[0:1, 1:2], warm[0:1, 0:1], AF.Sqrt)
            nc.sync.dma_start(wout_d[:, :], warm[0:1, 0:64])
            psW = psq.tile([128, 1024], F32, tag="q")
            for _ in range(N_WARMUP):
                nc.tensor.matmul(psW[:, 0:384], warm[:, 0:128], warm[:, 0:384],
                                 start=True, stop=True)

            # -- phase 1: diff -> square -> coord-sum -> sqrt => distT fp16
            psD1 = psx.tile([128, 512], F32, tag="x")
            nc.tensor.matmul(psD1[:, :], a1, data, start=True, stop=True)
            psD2 = psx.tile([128, 512], F32, tag="x")
            nc.tensor.matmul(psD2[0:64, :], a2, data, start=True, stop=True)

            sq1 = wpool.tile([128, 512], F16, tag="sq1")
            sq2 = wpool.tile([64, 512], F16, tag="sq2")
            nc.scalar.activation(sq1[:, :], psD1[:, :], AF.Square)
            nc.scalar.activation(sq2[:, :], psD2[0:64, :], AF.Square)

            psE1 = psx.tile([64, 256], F32, tag="e")
            nc.tensor.matmul(psE1[:, :], cs1, sq1[:, 0:256],
                             start=True, stop=False)
            nc.tensor.matmul(psE1[:, :], cs2, sq2[:, 0:256],
                             start=False, stop=True)
            psE2 = psx.tile([64, 256], F32, tag="e")
            nc.tensor.matmul(psE2[:, :], cs1, sq1[:, 256:512],
                             start=True, stop=False)
            nc.tensor.matmul(psE2[:, :], cs2, sq2[:, 256:512],
                             start=False, stop=True)

            distT = wpool.tile([64, 512], F16, tag="distT")
            nc.scalar.activation(distT[:, 0:256], psE1[:, :], AF.Sqrt)
            nc.scalar.activation(distT[:, 256:512], psE2[:, :], AF.Sqrt)

            # -- L1 in sample-halves: per half, both orderings side-by-side
            # in one single-bank tile; fused strided min reduce per half
            m_t = wpool.tile([112, 512], F16, tag="m")
            psL1 = psx.tile([128, 512], F32, tag="x")
            nc.tensor.matmul(psL1[0:112, 0:256], l1a, distT[:, 0:256],
                             start=True, stop=True)
            nc.tensor.matmul(psL1[0:112, 256:512], l1b, distT[:, 0:256],
                             start=True, stop=True)
            nc.vector.tensor_reduce(
                m_t[:, 0:256],
                psL1[0:112, :].rearrange("p (k j) -> p j k", k=2),
                axis=mybir.AxisListType.X, op=ALU.min)
            psL2 = psx.tile([128, 512], F32, tag="x")
            nc.tensor.matmul(psL2[0:112, 0:256], l1a, distT[:, 256:512],
                             start=True, stop=True)
            nc.tensor.matmul(psL2[0:112, 256:512], l1b, distT[:, 256:512],
                             start=True, stop=True)
            nc.vector.tensor_reduce(
                m_t[:, 256:512],
                psL2[0:112, :].rearrange("p (k j) -> p j k", k=2),
                axis=mybir.AxisListType.X, op=ALU.min)

            # -- L2 + L3 per 128-sample chunk
            minall = wpool.tile([128, 560], F16, tag="minall")
            msum = wpool.tile([128, 280], F16, tag="msum")
            loss = wpool.tile([128, 64], F32, tag="loss")
            nc.vector.memset(loss[:, :], 0.0)

            for c in range(N_CHUNKS):
                sl = slice(c * CHUNK, (c + 1) * CHUNK)
                psQ = psq.tile([128, 1024], F32, tag="q")
                nc.tensor.matmul(psQ[:, 0:420], m_t[:, sl], l2a,
                                 start=True, stop=True)
                nc.tensor.matmul(psQ[:, 512:932], m_t[:, sl], l2b,
                                 start=True, stop=True)

                psv = (psQ[:, :].rearrange("p (k j) -> p k j", k=2)[:, :, 0:420]
                       .rearrange("p k (t s) -> p k t s", s=6))
                mout = minall[:, c * 140:(c + 1) * 140]
                nc.vector.tensor_reduce(
                    mout, psv, axis=mybir.AxisListType.X, op=ALU.min)

                # A+B add (Pool for c0-c2; DVE for c3 to shorten the tail)
                eng = nc.gpsimd if c < 3 else nc.vector
                eng.tensor_tensor(
                    msum[:, c * 70:(c + 1) * 70],
                    minall[:, c * 140:c * 140 + 70],
                    minall[:, c * 140 + 70:c * 140 + 140],
                    op=ALU.add)

            # one fused min-over-70 for all four chunks
            nc.vector.tensor_reduce(
                loss[:, 0:4], msum[:, :].rearrange("p (c t) -> p c t", c=4),
                axis=mybir.AxisListType.X, op=ALU.min)

            # -- finale: fp32 transpose [128,64] -> [64,128], one 2KB DMA
            psT = psx.tile([64, 256], F32, tag="e")
            nc.tensor.transpose(psT[:, 0:128], loss[:, :], ID32[:, :])
            outb = wpool.tile([4, 128], F32, tag="outb")
            nc.vector.tensor_copy(outb[:, :], psT[0:4, 0:128])
            nc.sync.dma_start(
                out_d[:].rearrange("(c p) -> c p", p=128), outb[:, :])

    nc.compile()
    return nc


_NC = None


def _get_nc():
    global _NC
    if _NC is None:
        _NC = build_nc()
    return _NC


def kernel(pred_corners: np.ndarray, target_corners: np.ndarray) -> np.ndarray:
    from concourse.bass_utils import run_bass_kernel_spmd

    nc = _get_nc()
    in_maps = make_in_maps(pred_corners, target_corners)
    res = run_bass_kernel_spmd(nc, in_maps, core_ids=list(range(N_CORES)))
    return np.concatenate([res.results[k]["out"] for k in range(N_CORES)])
